# revision 12
# baseline (speedup 1.0000x reference)
"""Trainium2 Bass kernel: PVT-style cross-attention with spatial reduction."""
import sys
sys.path.insert(0, "/opt/trn_rl_repo")
from contextlib import ExitStack

import concourse.bass as bass
import concourse.tile as tile
from concourse import bacc, mybir, masks

dt = mybir.dt
AF = mybir.ActivationFunctionType
ALU = mybir.AluOpType
f32 = dt.float32
f32r = dt.float32r

N = 4096          # query tokens per core
C = 512           # model dim
CC = 512          # cross dim
NH = 8            # heads
HD = 64           # head dim
INNER = NH * HD   # 512
NK = 1024         # key tokens after spatial reduction
SCALE = HD ** -0.5
EPS = 1e-5

NS = N // 512     # 8 n-splits of 512 queries


def ts(i, s):
    return bass.ts(i, s)


def build_core_program(nsplits=NS):
    """Build the single-core Bass program. Returns nc."""
    nc = bacc.Bacc("TRN2", target_bir_lowering=False, debug=False)

    # ---- DRAM I/O --------------------------------------------------------
    x_d = nc.dram_tensor("x", (N, C), f32, kind="ExternalInput").ap()
    y_d = nc.dram_tensor("y", (4096, CC), f32, kind="ExternalInput").ap()
    wq_d = nc.dram_tensor("Wq", (C, INNER), f32r, kind="ExternalInput").ap()
    wkv_d = nc.dram_tensor("Wkv", (CC, 2 * INNER), f32r, kind="ExternalInput").ap()
    wproj_d = nc.dram_tensor("Wproj", (INNER, C), f32r, kind="ExternalInput").ap()
    bproj_d = nc.dram_tensor("bproj", (C,), f32, kind="ExternalInput").ap()
    gcross_d = nc.dram_tensor("g_cross", (CC,), f32, kind="ExternalInput").ap()
    bcross_d = nc.dram_tensor("b_cross", (CC,), f32, kind="ExternalInput").ap()
    srw_d = nc.dram_tensor("sr_w", (2, 2, CC, CC), f32r, kind="ExternalInput").ap()
    srb_d = nc.dram_tensor("sr_b", (CC,), f32, kind="ExternalInput").ap()
    gsr_d = nc.dram_tensor("g_sr", (CC,), f32, kind="ExternalInput").ap()
    bsr_d = nc.dram_tensor("b_sr", (CC,), f32, kind="ExternalInput").ap()
    out_d = nc.dram_tensor("out", (N, C), f32, kind="ExternalOutput").ap()

    with tile.TileContext(nc) as tc, ExitStack() as octx:
        # persistent pools
        wpool = octx.enter_context(tc.tile_pool(name="weights", bufs=1))
        kvpool = octx.enter_context(tc.tile_pool(name="kv", bufs=1))
        ps = octx.enter_context(tc.tile_pool(name="ps", bufs=2, space="PSUM"))

        # ---- constants / weights ----------------------------------------
        ident = wpool.tile([128, 128], f32, tag="ident")
        masks.make_identity(nc, ident[:])
        epst = wpool.tile([128, 1], f32, tag="eps")
        nc.vector.memset(epst[:], EPS)
        onesf = wpool.tile([128, 8], f32, tag="onesf")
        nc.vector.memset(onesf[:], 1.0)
        ones = wpool.tile([128, 1], f32r, tag="ones")
        nc.vector.tensor_copy(ones[:], onesf[:, 0:1])

        wq = [wpool.tile([128, INNER], f32r, tag=f"wq{c}", name=f"wq{c}") for c in range(4)]
        wproj = [wpool.tile([128, C], f32r, tag=f"wp{c}", name=f"wp{c}") for c in range(4)]
        for c in range(4):
            nc.sync.dma_start(wq[c][:], wq_d[ts(c, 128), :])
            nc.sync.dma_start(wproj[c][:], wproj_d[ts(c, 128), :])

        # per-channel vectors as [128, 4] (chunk-major free dim)
        def chanvec(name, src):
            t = wpool.tile([128, 4], f32, tag=name, name=name)
            nc.sync.dma_start(t[:], src.rearrange("(c p) -> p c", p=128))
            return t

        gcross = chanvec("gcross", gcross_d)
        bcross = chanvec("bcross", bcross_d)
        gsr = chanvec("gsr", gsr_d)
        bsr = chanvec("bsr", bsr_d)
        srb = chanvec("srb", srb_d)

        # bproj broadcast to all partitions
        bproj_row = wpool.tile([1, C], f32, tag="bprow")
        nc.sync.dma_start(bproj_row[:], bproj_d.rearrange("(a c) -> a c", a=1))
        bproj_b = wpool.tile([128, C], f32, tag="bpb")
        nc.gpsimd.partition_broadcast(bproj_b[:], bproj_row[:])

        # persistent context tensors
        kT = [kvpool.tile([128, NK], f32r, tag=f"kT{c}", name=f"kT{c}") for c in range(4)]
        v_aug = [kvpool.tile([128, NH * (HD + 1)], f32r, tag=f"va{m}", name=f"va{m}")
                 for m in range(NK // 128)]

        # =================================================================
        # Stage B prep helpers (pipelined): x load + transpose + qT.
        # prep(0) is emitted before stage A so PE has dense work while the
        # y-LN chain (DVE/ACT) runs; prep(ns+1) and proj(ns-1) are
        # interleaved into attention(ns) blocks as PE filler.
        # =================================================================
        def new_prep_tiles():
            xT = [ppool.tile([128, 512], f32r, tag=f"xT{c}", name=f"xT{c}",
                             bufs=1) for c in range(4)]
            qT = [ppool.tile([128, 512], f32r, tag=f"qT{c}", name=f"qT{c}",
                             bufs=2) for c in range(4)]
            return xT, qT

        def emit_prep_dma(ns):
            xts = []
            for r in range(4):
                xt = ppool.tile([128, C], f32, tag="xload", name="xload",
                                bufs=4)
                nc.sync.dma_start(xt[:], x_d[ts(ns * 4 + r, 128), :])
                xts.append(xt)
            return xts

        def emit_prep_transpose(r, xt, xT):
            for c in range(4):
                pt = ps.tile([128, 128], f32, tag="mm", name="pt", bufs=1)
                nc.tensor.transpose(pt[:], xt[:, ts(c, 128)], ident[:])
                nc.vector.tensor_copy(xT[c][:, ts(r, 128)], pt[:])

        def emit_prep_q(xT, qT):
            for icn in range(4):
                pq = ps.tile([128, 512], f32, tag="mm", name="pq", bufs=1)
                for ci in range(4):
                    nc.tensor.matmul(pq[:], wq[ci][:, ts(icn, 128)],
                                     xT[ci][:], start=(ci == 0),
                                     stop=(ci == 3))
                nc.vector.tensor_copy(qT[icn][:], pq[:])

        def emit_proj_block(outT_src, ns_prev, r):
            pf = ps.tile([128, 512], f32, tag="mm", name="pf", bufs=1)
            for icn in range(4):
                nc.tensor.matmul(pf[:], outT_src[icn][:, ts(r, 128)],
                                 wproj[icn][:], start=(icn == 0),
                                 stop=(icn == 3))
            fin = bpool.tile([128, C], f32, tag="fin", name="fin")
            nc.vector.tensor_tensor(fin[:], pf[:], bproj_b[:], op=ALU.add)
            nc.sync.dma_start(out_d[ts(ns_prev * 4 + r, 128), :], fin[:])

        # prefetch x(0) before stage A so prep(0) starts instantly after A
        xpre = octx.enter_context(tc.tile_pool(name="xpre", bufs=4))
        xts0 = []
        for r in range(4):
            xt = xpre.tile([128, C], f32, tag="xload0", name="xload0", bufs=4)
            nc.sync.dma_start(xt[:], x_d[ts(r, 128), :])
            xts0.append(xt)

        # =================================================================
        # Stage A: context prep (y -> LN -> conv -> LN -> kv)
        # =================================================================
        with ExitStack() as actx:
            apool = actx.enter_context(tc.tile_pool(name="stageA", bufs=2))
            a1pool = actx.enter_context(tc.tile_pool(name="stageA1", bufs=1))

            wkv = [a1pool.tile([128, 2 * INNER], f32r, tag=f"wkv{c}", name=f"wkv{c}")
                   for c in range(4)]
            for c in range(4):
                nc.sync.dma_start(wkv[c][:], wkv_d[ts(c, 128), :])
            srw = {}
            for di in range(2):
                for dj in range(2):
                    for c in range(4):
                        t = a1pool.tile([128, CC], f32r, tag=f"srw{di}{dj}{c}", name=f"srw{di}{dj}{c}")
                        nc.sync.dma_start(t[:], srw_d[di, dj, ts(c, 128), :])
                        srw[(di, dj, c)] = t

            # x_conv accumulates the (biased) conv output in T layout
            x_raw = [a1pool.tile([128, NK], f32r, tag=f"xr{c}", name=f"xr{c}") for c in range(4)]

            # ---- A1+A2 fused per output-row group ------------------------
            for gg in range(4):
                # transpose this group's 1024 input pixels into yT_g
                ytg = [apool.tile([128, 1024], f32r, tag=f"ytg{c}", name=f"ytg{c}", bufs=2)
                       for c in range(4)]
                for t8 in range(8):
                    trow = gg * 8 + t8       # y tile index (128 pixels each)
                    yt = apool.tile([128, CC], f32, tag="yload", bufs=4)
                    nc.sync.dma_start(yt[:], y_d[ts(trow, 128), :])
                    st = apool.tile([128, 6], f32, tag="bnst", bufs=4)
                    ag = apool.tile([128, 2], f32, tag="bnag", bufs=4)
                    nc.vector.bn_stats(st[:], yt[:])
                    nc.vector.bn_aggr(ag[:], st[:])
                    rstd = apool.tile([128, 1], f32, tag="rstd", bufs=4)
                    nc.scalar.activation(rstd[:], ag[:, 1:2], AF.Sqrt,
                                         bias=epst[:, 0:1])
                    nc.vector.reciprocal_approx_fast(rstd[:], rstd[:])
                    nmr = apool.tile([128, 1], f32, tag="nmr", bufs=4)
                    nc.vector.tensor_scalar(nmr[:], ag[:, 0:1], rstd[:, 0:1],
                                            -1.0, op0=ALU.mult, op1=ALU.mult)
                    yln = apool.tile([128, CC], f32, tag="yln", bufs=4)
                    nc.scalar.activation(yln[:], yt[:], AF.Identity,
                                         bias=nmr[:, 0:1],
                                         scale=rstd[:, 0:1])
                    for c in range(4):
                        pt = ps.tile([128, 128], f32, tag="att", bufs=3)
                        nc.tensor.transpose(pt[:], yln[:, ts(c, 128)], ident[:])
                        # fuse g_cross/b_cross (per-channel) into evacuation;
                        # split between ACT and DVE to balance engines
                        if c % 2 == 0:
                            nc.scalar.activation(
                                ytg[c][:, ts(t8, 128)], pt[:], AF.Identity,
                                bias=bcross[:, c:c + 1],
                                scale=gcross[:, c:c + 1])
                        else:
                            nc.vector.tensor_scalar(
                                ytg[c][:, ts(t8, 128)], pt[:],
                                gcross[:, c:c + 1], bcross[:, c:c + 1],
                                op0=ALU.mult, op1=ALU.add)

                # conv over this group: 256 output pixels
                for co in range(4):
                    pc = ps.tile([128, 256], f32, tag="att", bufs=3)
                    first = True
                    for ci in range(4):
                        view = ytg[ci][:].rearrange(
                            "p (i two j s) -> p i two j s",
                            i=8, two=2, j=32, s=2)
                        for di in range(2):
                            for dj in range(2):
                                g = view[:, :, di:di + 1, :, dj:dj + 1]
                                nc.tensor.matmul(
                                    pc[:],
                                    srw[(di, dj, ci)][:, ts(co, 128)],
                                    g,
                                    start=first,
                                    stop=(ci == 3 and di == 1 and dj == 1))
                                first = False
                    # + sr_b, into x_raw
                    nc.vector.tensor_scalar(
                        x_raw[co][:, ts(gg, 256)], pc[:], srb[:, co:co + 1],
                        None, op0=ALU.add)

            # ---- A3: LN_sr over x_raw (per-pixel over channels) ----------
            ssum = apool.tile([1, NK], f32, tag="ssum", bufs=1)
            ssq = apool.tile([1, NK], f32, tag="ssq", bufs=1)
            for sp in range(2):
                p_sum = ps.tile([1, 512], f32, tag="att", bufs=3)
                p_sq = ps.tile([1, 512], f32, tag="att", bufs=3)
                for ci in range(4):
                    nc.tensor.matmul(p_sum[:], ones[:],
                                     x_raw[ci][:, ts(sp, 512)],
                                     start=(ci == 0), stop=(ci == 3))
                for ci in range(4):
                    sq = apool.tile([128, 512], f32r, tag="sq", bufs=1)
                    nc.scalar.activation(sq[:], x_raw[ci][:, ts(sp, 512)],
                                         AF.Square)
                    nc.tensor.matmul(p_sq[:], ones[:],
                                     sq[:],
                                     start=(ci == 0), stop=(ci == 3))
                nc.vector.tensor_copy(ssum[:, ts(sp, 512)], p_sum[:])
                nc.vector.tensor_copy(ssq[:, ts(sp, 512)], p_sq[:])

            # in-place stats: ssum -> mean -> -mean*rstd ; ssq -> var ; sc1 -> rstd
            sc1 = apool.tile([1, NK], f32, tag="sc1", bufs=1)
            nc.vector.tensor_scalar(ssum[:], ssum[:], 1.0 / CC, None,
                                    op0=ALU.mult)
            nc.vector.tensor_scalar(ssq[:], ssq[:], 1.0 / CC, None,
                                    op0=ALU.mult)
            nc.vector.tensor_tensor(sc1[:], ssum[:], ssum[:], op=ALU.mult)
            nc.vector.tensor_tensor(ssq[:], ssq[:], sc1[:], op=ALU.subtract)
            nc.scalar.activation(sc1[:], ssq[:], AF.Sqrt, bias=epst[0:1, 0:1])
            nc.vector.reciprocal_approx_fast(sc1[:], sc1[:])
            nc.vector.scalar_tensor_tensor(ssum[:], ssum[:], -1.0, sc1[:],
                                           op0=ALU.mult, op1=ALU.mult)
            rb = a1pool.tile([128, NK], f32, tag="rb")
            nb = a1pool.tile([128, NK], f32, tag="nb")
            nc.gpsimd.partition_broadcast(rb[:], sc1[:])
            nc.gpsimd.partition_broadcast(nb[:], ssum[:])
            for ci in range(4):
                nc.vector.tensor_tensor(x_raw[ci][:], x_raw[ci][:], rb[:],
                                        op=ALU.mult)
                nc.vector.tensor_tensor(x_raw[ci][:], x_raw[ci][:], nb[:],
                                        op=ALU.add)
                nc.vector.tensor_scalar(x_raw[ci][:], x_raw[ci][:],
                                        gsr[:, ci:ci + 1], bsr[:, ci:ci + 1],
                                        op0=ALU.mult, op1=ALU.add)

            # ---- A4: kv projections -------------------------------------
            # kT[i_chunk][128, NK]
            for icn in range(4):
                for msp in range(2):
                    pk = ps.tile([128, 512], f32, tag="att", bufs=3)
                    for ci in range(4):
                        nc.tensor.matmul(
                            pk[:], wkv[ci][:, ts(icn, 128)],
                            x_raw[ci][:, ts(msp, 512)],
                            start=(ci == 0), stop=(ci == 3))
                    nc.vector.tensor_copy(kT[icn][:, ts(msp, 512)], pk[:])
            # v_aug[m_chunk][128, 8*65], with ones in col 65h+64
            for mc in range(NK // 128):
                pv = ps.tile([128, 512], f32, tag="att", bufs=3)
                for ci in range(4):
                    nc.tensor.matmul(
                        pv[:], x_raw[ci][:, ts(mc, 128)],
                        wkv[ci][:, INNER:2 * INNER],
                        start=(ci == 0), stop=(ci == 3))
                va_view = v_aug[mc][:].rearrange("p (h e) -> p h e", e=HD + 1)
                nc.vector.tensor_copy(
                    va_view[:, :, 0:HD],
                    pv[:].rearrange("p (h e) -> p h e", e=HD))
                nc.vector.tensor_copy(
                    va_view[:, :, HD:HD + 1],
                    onesf[:].rearrange("p (h o) -> p h o", o=1))

        # =================================================================
        # Stage B (pipelined): attention(ns) with interleaved PE filler
        # =================================================================
        ppool = octx.enter_context(tc.tile_pool(name="prep", bufs=1))
        bpool = octx.enter_context(tc.tile_pool(name="stageB", bufs=2))
        xT0, qT0 = new_prep_tiles()
        for r in range(4):
            emit_prep_transpose(r, xts0[r], xT0)
        emit_prep_q(xT0, qT0)
        qT_list = [None] * nsplits
        qT_list[0] = qT0
        outT_list = [None] * nsplits
        for ns in range(nsplits):
            qT = qT_list[ns]
            outT = [bpool.tile([128, 512], f32r, tag=f"oT{c}", name=f"oT{c}")
                    for c in range(4)]
            # build the PE-filler queue for this split: proj of ns-1 and
            # prep of ns+1, emitted just-in-time between attention steps
            filler = []
            if ns >= 1:
                for r in range(4):
                    filler.append(('proj', r))
            if ns + 1 < nsplits:
                xTn, qTn = new_prep_tiles()
                qT_list[ns + 1] = qTn
                xts = emit_prep_dma(ns + 1)
                for r in range(4):
                    filler.append(('ptr', r))
                filler.append(('pq', 0))
            nf = len(filler)
            fidx = 0
            it = 0
            for hp in range(4):
                po = [ps.tile([HD + 1, 512], f32, tag="att", name=f"po{e_}",
                               bufs=3) for e_ in range(2)]
                for mc in range(NK // 128):
                    pss = ps.tile([128, 1024], f32, tag="sc")
                    for e in range(2):
                        nc.tensor.matmul(
                            pss[:, ts(e, 512)],
                            kT[hp][ts(e, 64), ts(mc, 128)],
                            qT[hp][ts(e, 64), :],
                            start=True, stop=True,
                            tile_position=(64 * e, 0))
                    ex = bpool.tile([128, 1024], f32r, tag="ex")
                    nc.scalar.activation(ex[:], pss[:], AF.Exp, scale=SCALE)
                    # just-in-time PE filler while ACT computes the exp
                    while fidx < nf and fidx * 32 <= it * nf:
                        kind, r = filler[fidx]
                        if kind == 'proj':
                            emit_proj_block(outT_list[ns - 1], ns - 1, r)
                        elif kind == 'ptr':
                            emit_prep_transpose(r, xts[r], xTn)
                        else:
                            emit_prep_q(xTn, qTn)
                        fidx += 1
                    it += 1
                    for e in range(2):
                        h = 2 * hp + e
                        nc.tensor.matmul(
                            po[e][:],
                            v_aug[mc][:, ts(h, HD + 1)],
                            ex[:, ts(e, 512)],
                            start=(mc == 0), stop=(mc == NK // 128 - 1))
                for e in range(2):
                    # evacuate the accumulator immediately so the PSUM slot
                    # frees for the next head pair; normalize off the copy
                    poc = bpool.tile([HD + 1, 512], f32, tag="poc")
                    nc.vector.tensor_copy(poc[:], po[e][:])
                    dsb = bpool.tile([1, 512], f32, tag="dsb")
                    nc.vector.tensor_copy(dsb[:], poc[HD:HD + 1, :])
                    recip = bpool.tile([1, 512], f32, tag="recip")
                    nc.vector.reciprocal_approx_fast(recip[:], dsb[:])
                    bc = bpool.tile([HD, 512], f32, tag="bc")
                    nc.gpsimd.partition_broadcast(bc[:], recip[:])
                    nc.vector.tensor_tensor(outT[hp][ts(e, 64), :],
                                            poc[0:HD, :], bc[:],
                                            op=ALU.mult)
            outT_list[ns] = outT
        for r in range(4):
            emit_proj_block(outT_list[nsplits - 1], nsplits - 1, r)

    nc.compile()
    return nc


# ---------------------------------------------------------------------------
# SPMD entry point: full inputs in, full output out. One batch element per
# NeuronCore (B=8 over 8 cores), no collectives.
# ---------------------------------------------------------------------------
import numpy as np

_NC_CACHE = None


def _get_nc():
    global _NC_CACHE
    if _NC_CACHE is None:
        _NC_CACHE = build_core_program()
    return _NC_CACHE


_SHARED = ["Wq", "Wkv", "Wproj", "bproj", "g_cross", "b_cross", "sr_w",
           "sr_b", "g_sr", "b_sr"]


def _run(inputs, trace=False, trace_kwargs=None):
    from concourse.bass_utils import run_bass_kernel_spmd
    nc = _get_nc()
    shared = {k: np.ascontiguousarray(np.asarray(inputs[k], dtype=np.float32))
              for k in _SHARED}
    x = np.asarray(inputs["x"], dtype=np.float32)
    y = np.asarray(inputs["y"], dtype=np.float32)
    n_cores = 8
    in_maps = []
    for i in range(n_cores):
        m = dict(shared)
        m["x"] = np.ascontiguousarray(x[i])
        m["y"] = np.ascontiguousarray(y[i])
        in_maps.append(m)
    kw = {}
    if trace:
        kw["trace"] = True
        if trace_kwargs:
            kw.update(trace_kwargs)
    res = run_bass_kernel_spmd(nc, in_maps, list(range(n_cores)), **kw)
    out = np.stack([res.results[i]["out"] for i in range(n_cores)], axis=0)
    return out, res


def kernel(**inputs):
    out, _ = _run(inputs)
    return out


# revision 13
# speedup vs baseline: 1.0413x; 1.0413x over previous
"""Trainium2 Bass kernel: PVT-style cross-attention with spatial reduction."""
import sys
sys.path.insert(0, "/opt/trn_rl_repo")
from contextlib import ExitStack

import concourse.bass as bass
import concourse.tile as tile
from concourse import bacc, mybir, masks

dt = mybir.dt
AF = mybir.ActivationFunctionType
ALU = mybir.AluOpType
f32 = dt.float32
f32r = dt.float32r

N = 4096          # query tokens per core
C = 512           # model dim
CC = 512          # cross dim
NH = 8            # heads
HD = 64           # head dim
INNER = NH * HD   # 512
NK = 1024         # key tokens after spatial reduction
SCALE = HD ** -0.5
EPS = 1e-5

NS = N // 512     # 8 n-splits of 512 queries


def ts(i, s):
    return bass.ts(i, s)


def build_core_program(nsplits=NS):
    """Build the single-core Bass program. Returns nc."""
    nc = bacc.Bacc("TRN2", target_bir_lowering=False, debug=False)

    # ---- DRAM I/O --------------------------------------------------------
    x_d = nc.dram_tensor("x", (N, C), f32, kind="ExternalInput").ap()
    y_d = nc.dram_tensor("y", (4096, CC), f32, kind="ExternalInput").ap()
    wq_d = nc.dram_tensor("Wq", (C, INNER), f32r, kind="ExternalInput").ap()
    wkv_d = nc.dram_tensor("Wkv", (CC, 2 * INNER), f32r, kind="ExternalInput").ap()
    wproj_d = nc.dram_tensor("Wproj", (INNER, C), f32r, kind="ExternalInput").ap()
    bproj_d = nc.dram_tensor("bproj", (C,), f32, kind="ExternalInput").ap()
    gcross_d = nc.dram_tensor("g_cross", (CC,), f32, kind="ExternalInput").ap()
    bcross_d = nc.dram_tensor("b_cross", (CC,), f32, kind="ExternalInput").ap()
    srw_d = nc.dram_tensor("sr_w", (2, 2, CC, CC), f32r, kind="ExternalInput").ap()
    srb_d = nc.dram_tensor("sr_b", (CC,), f32, kind="ExternalInput").ap()
    gsr_d = nc.dram_tensor("g_sr", (CC,), f32, kind="ExternalInput").ap()
    bsr_d = nc.dram_tensor("b_sr", (CC,), f32, kind="ExternalInput").ap()
    out_d = nc.dram_tensor("out", (N, C), f32, kind="ExternalOutput").ap()

    with tile.TileContext(nc) as tc, ExitStack() as octx:
        # persistent pools
        wpool = octx.enter_context(tc.tile_pool(name="weights", bufs=1))
        kvpool = octx.enter_context(tc.tile_pool(name="kv", bufs=1))
        ps = octx.enter_context(tc.tile_pool(name="ps", bufs=2, space="PSUM"))

        # ---- constants / weights ----------------------------------------
        ident = wpool.tile([128, 128], f32, tag="ident")
        masks.make_identity(nc, ident[:])
        epst = wpool.tile([128, 1], f32, tag="eps")
        nc.vector.memset(epst[:], EPS)
        onesf = wpool.tile([128, 8], f32, tag="onesf")
        nc.vector.memset(onesf[:], 1.0)
        ones = wpool.tile([128, 1], f32r, tag="ones")
        nc.vector.tensor_copy(ones[:], onesf[:, 0:1])

        wq = [wpool.tile([128, INNER], f32r, tag=f"wq{c}", name=f"wq{c}") for c in range(4)]
        wproj = [wpool.tile([128, C], f32r, tag=f"wp{c}", name=f"wp{c}") for c in range(4)]

        def emit_wq_wproj_dmas():
            for c in range(4):
                nc.sync.dma_start(wq[c][:], wq_d[ts(c, 128), :])
                nc.sync.dma_start(wproj[c][:], wproj_d[ts(c, 128), :])

        # per-channel vectors as [128, 4] (chunk-major free dim)
        def chanvec(name, src):
            t = wpool.tile([128, 4], f32, tag=name, name=name)
            nc.sync.dma_start(t[:], src.rearrange("(c p) -> p c", p=128))
            return t

        gcross = chanvec("gcross", gcross_d)
        bcross = chanvec("bcross", bcross_d)
        gsr = chanvec("gsr", gsr_d)
        bsr = chanvec("bsr", bsr_d)
        srb = chanvec("srb", srb_d)

        # bproj broadcast to all partitions
        bproj_row = wpool.tile([1, C], f32, tag="bprow")
        nc.sync.dma_start(bproj_row[:], bproj_d.rearrange("(a c) -> a c", a=1))
        bproj_b = wpool.tile([128, C], f32, tag="bpb")
        nc.gpsimd.partition_broadcast(bproj_b[:], bproj_row[:])

        # persistent context tensors
        kT = [kvpool.tile([128, NK], f32r, tag=f"kT{c}", name=f"kT{c}") for c in range(4)]
        v_aug = [kvpool.tile([128, NH * (HD + 1)], f32r, tag=f"va{m}", name=f"va{m}")
                 for m in range(NK // 128)]

        # =================================================================
        # Stage B prep helpers (pipelined): x load + transpose + qT.
        # prep(0) is emitted before stage A so PE has dense work while the
        # y-LN chain (DVE/ACT) runs; prep(ns+1) and proj(ns-1) are
        # interleaved into attention(ns) blocks as PE filler.
        # =================================================================
        def new_prep_tiles():
            xT = [ppool.tile([128, 512], f32r, tag=f"xT{c}", name=f"xT{c}",
                             bufs=1) for c in range(4)]
            qT = [ppool.tile([128, 512], f32r, tag=f"qT{c}", name=f"qT{c}",
                             bufs=2) for c in range(4)]
            return xT, qT

        def emit_prep_dma(ns):
            xts = []
            for r in range(4):
                xt = ppool.tile([128, C], f32, tag="xload", name="xload",
                                bufs=4)
                nc.sync.dma_start(xt[:], x_d[ts(ns * 4 + r, 128), :])
                xts.append(xt)
            return xts

        def emit_prep_transpose(r, xt, xT):
            for c in range(4):
                pt = ps.tile([128, 128], f32, tag="mm", name="pt", bufs=1)
                nc.tensor.transpose(pt[:], xt[:, ts(c, 128)], ident[:])
                nc.vector.tensor_copy(xT[c][:, ts(r, 128)], pt[:])

        def emit_prep_q(xT, qT):
            for icn in range(4):
                pq = ps.tile([128, 512], f32, tag="mm", name="pq", bufs=1)
                for ci in range(4):
                    nc.tensor.matmul(pq[:], wq[ci][:, ts(icn, 128)],
                                     xT[ci][:], start=(ci == 0),
                                     stop=(ci == 3))
                nc.vector.tensor_copy(qT[icn][:], pq[:])

        def emit_proj_block(outT_src, ns_prev, r):
            pf = ps.tile([128, 512], f32, tag="mm", name="pf", bufs=1)
            for icn in range(4):
                nc.tensor.matmul(pf[:], outT_src[icn][:, ts(r, 128)],
                                 wproj[icn][:], start=(icn == 0),
                                 stop=(icn == 3))
            fin = bpool.tile([128, C], f32, tag="fin", name="fin")
            nc.vector.tensor_tensor(fin[:], pf[:], bproj_b[:], op=ALU.add)
            nc.sync.dma_start(out_d[ts(ns_prev * 4 + r, 128), :], fin[:])

        # prefetch x(0) before stage A so prep(0) starts instantly after A
        xpre = octx.enter_context(tc.tile_pool(name="xpre", bufs=4))
        xts0 = [xpre.tile([128, C], f32, tag="xload0", name="xload0", bufs=4)
                for r in range(4)]

        def emit_xpre_dmas():
            for r in range(4):
                nc.sync.dma_start(xts0[r][:], x_d[ts(r, 128), :])

        # =================================================================
        # Stage A: context prep (y -> LN -> conv -> LN -> kv)
        # =================================================================
        with ExitStack() as actx:
            apool = actx.enter_context(tc.tile_pool(name="stageA", bufs=2))
            a1pool = actx.enter_context(tc.tile_pool(name="stageA1", bufs=1))

            wkv = [a1pool.tile([128, 2 * INNER], f32r, tag=f"wkv{c}", name=f"wkv{c}")
                   for c in range(4)]
            srw = {}
            for di in range(2):
                for dj in range(2):
                    for c in range(4):
                        t = a1pool.tile([128, CC], f32r, tag=f"srw{di}{dj}{c}", name=f"srw{di}{dj}{c}")
                        srw[(di, dj, c)] = t

            def emit_wkv_dmas():
                for c in range(4):
                    nc.sync.dma_start(wkv[c][:], wkv_d[ts(c, 128), :])

            def emit_srw_dmas():
                for di in range(2):
                    for dj in range(2):
                        for c in range(4):
                            nc.sync.dma_start(srw[(di, dj, c)][:],
                                              srw_d[di, dj, ts(c, 128), :])

            # x_conv accumulates the (biased) conv output in T layout
            x_raw = [a1pool.tile([128, NK], f32r, tag=f"xr{c}", name=f"xr{c}") for c in range(4)]

            # ---- A1+A2 fused per output-row group ------------------------
            for gg in range(4):
                # transpose this group's 1024 input pixels into yT_g
                ytg = [apool.tile([128, 1024], f32r, tag=f"ytg{c}", name=f"ytg{c}", bufs=2)
                       for c in range(4)]
                for t8 in range(8):
                    trow = gg * 8 + t8       # y tile index (128 pixels each)
                    yt = apool.tile([128, CC], f32, tag="yload", bufs=4)
                    nc.sync.dma_start(yt[:], y_d[ts(trow, 128), :])
                    st = apool.tile([128, 6], f32, tag="bnst", bufs=4)
                    ag = apool.tile([128, 2], f32, tag="bnag", bufs=4)
                    nc.vector.bn_stats(st[:], yt[:])
                    nc.vector.bn_aggr(ag[:], st[:])
                    rstd = apool.tile([128, 1], f32, tag="rstd", bufs=4)
                    nc.scalar.activation(rstd[:], ag[:, 1:2], AF.Sqrt,
                                         bias=epst[:, 0:1])
                    nc.vector.reciprocal_approx_fast(rstd[:], rstd[:])
                    nmr = apool.tile([128, 1], f32, tag="nmr", bufs=4)
                    nc.vector.tensor_scalar(nmr[:], ag[:, 0:1], rstd[:, 0:1],
                                            -1.0, op0=ALU.mult, op1=ALU.mult)
                    yln = apool.tile([128, CC], f32, tag="yln", bufs=4)
                    nc.scalar.activation(yln[:], yt[:], AF.Identity,
                                         bias=nmr[:, 0:1],
                                         scale=rstd[:, 0:1])
                    for c in range(4):
                        pt = ps.tile([128, 128], f32, tag="att", bufs=3)
                        nc.tensor.transpose(pt[:], yln[:, ts(c, 128)], ident[:])
                        # fuse g_cross/b_cross (per-channel) into evacuation;
                        # split between ACT and DVE to balance engines
                        if c % 2 == 0:
                            nc.scalar.activation(
                                ytg[c][:, ts(t8, 128)], pt[:], AF.Identity,
                                bias=bcross[:, c:c + 1],
                                scale=gcross[:, c:c + 1])
                        else:
                            nc.vector.tensor_scalar(
                                ytg[c][:, ts(t8, 128)], pt[:],
                                gcross[:, c:c + 1], bcross[:, c:c + 1],
                                op0=ALU.mult, op1=ALU.add)

                # deferred weight DMAs: queue behind this group's y loads
                if gg == 0:
                    emit_srw_dmas()
                elif gg == 1:
                    emit_wkv_dmas()
                elif gg == 2:
                    emit_wq_wproj_dmas()
                    emit_xpre_dmas()

                # conv over this group: 256 output pixels
                for co in range(4):
                    pc = ps.tile([128, 256], f32, tag="att", bufs=3)
                    first = True
                    for ci in range(4):
                        view = ytg[ci][:].rearrange(
                            "p (i two j s) -> p i two j s",
                            i=8, two=2, j=32, s=2)
                        for di in range(2):
                            for dj in range(2):
                                g = view[:, :, di:di + 1, :, dj:dj + 1]
                                nc.tensor.matmul(
                                    pc[:],
                                    srw[(di, dj, ci)][:, ts(co, 128)],
                                    g,
                                    start=first,
                                    stop=(ci == 3 and di == 1 and dj == 1))
                                first = False
                    # + sr_b, into x_raw
                    nc.vector.tensor_scalar(
                        x_raw[co][:, ts(gg, 256)], pc[:], srb[:, co:co + 1],
                        None, op0=ALU.add)

            # ---- A3: LN_sr over x_raw (per-pixel over channels) ----------
            ssum = apool.tile([1, NK], f32, tag="ssum", bufs=1)
            ssq = apool.tile([1, NK], f32, tag="ssq", bufs=1)
            for sp in range(2):
                p_sum = ps.tile([1, 512], f32, tag="att", bufs=3)
                p_sq = ps.tile([1, 512], f32, tag="att", bufs=3)
                for ci in range(4):
                    nc.tensor.matmul(p_sum[:], ones[:],
                                     x_raw[ci][:, ts(sp, 512)],
                                     start=(ci == 0), stop=(ci == 3))
                for ci in range(4):
                    sq = apool.tile([128, 512], f32r, tag="sq", bufs=2)
                    nc.scalar.activation(sq[:], x_raw[ci][:, ts(sp, 512)],
                                         AF.Square)
                    nc.tensor.matmul(p_sq[:], ones[:],
                                     sq[:],
                                     start=(ci == 0), stop=(ci == 3))
                nc.vector.tensor_copy(ssum[:, ts(sp, 512)], p_sum[:])
                nc.vector.tensor_copy(ssq[:, ts(sp, 512)], p_sq[:])

            # in-place stats: ssum -> mean -> -mean*rstd ; ssq -> var ; sc1 -> rstd
            sc1 = apool.tile([1, NK], f32, tag="sc1", bufs=1)
            nc.vector.tensor_scalar(ssum[:], ssum[:], 1.0 / CC, None,
                                    op0=ALU.mult)
            nc.vector.tensor_scalar(ssq[:], ssq[:], 1.0 / CC, None,
                                    op0=ALU.mult)
            nc.vector.tensor_tensor(sc1[:], ssum[:], ssum[:], op=ALU.mult)
            nc.vector.tensor_tensor(ssq[:], ssq[:], sc1[:], op=ALU.subtract)
            nc.scalar.activation(sc1[:], ssq[:], AF.Sqrt, bias=epst[0:1, 0:1])
            nc.vector.reciprocal_approx_fast(sc1[:], sc1[:])
            nc.vector.scalar_tensor_tensor(ssum[:], ssum[:], -1.0, sc1[:],
                                           op0=ALU.mult, op1=ALU.mult)
            rb = a1pool.tile([128, NK], f32, tag="rb")
            nb = a1pool.tile([128, NK], f32, tag="nb")
            nc.gpsimd.partition_broadcast(rb[:], sc1[:])
            nc.gpsimd.partition_broadcast(nb[:], ssum[:])
            for ci in range(4):
                nc.vector.tensor_tensor(x_raw[ci][:], x_raw[ci][:], rb[:],
                                        op=ALU.mult)
                nc.vector.tensor_tensor(x_raw[ci][:], x_raw[ci][:], nb[:],
                                        op=ALU.add)
                nc.vector.tensor_scalar(x_raw[ci][:], x_raw[ci][:],
                                        gsr[:, ci:ci + 1], bsr[:, ci:ci + 1],
                                        op0=ALU.mult, op1=ALU.add)

            # ---- A4: kv projections -------------------------------------
            # kT[i_chunk][128, NK]
            for icn in range(4):
                for msp in range(2):
                    pk = ps.tile([128, 512], f32, tag="att", bufs=3)
                    for ci in range(4):
                        nc.tensor.matmul(
                            pk[:], wkv[ci][:, ts(icn, 128)],
                            x_raw[ci][:, ts(msp, 512)],
                            start=(ci == 0), stop=(ci == 3))
                    nc.vector.tensor_copy(kT[icn][:, ts(msp, 512)], pk[:])
            # v_aug[m_chunk][128, 8*65], with ones in col 65h+64
            for mc in range(NK // 128):
                pv = ps.tile([128, 512], f32, tag="att", bufs=3)
                for ci in range(4):
                    nc.tensor.matmul(
                        pv[:], x_raw[ci][:, ts(mc, 128)],
                        wkv[ci][:, INNER:2 * INNER],
                        start=(ci == 0), stop=(ci == 3))
                va_view = v_aug[mc][:].rearrange("p (h e) -> p h e", e=HD + 1)
                nc.vector.tensor_copy(
                    va_view[:, :, 0:HD],
                    pv[:].rearrange("p (h e) -> p h e", e=HD))
                nc.vector.tensor_copy(
                    va_view[:, :, HD:HD + 1],
                    onesf[:].rearrange("p (h o) -> p h o", o=1))

        # =================================================================
        # Stage B (pipelined): attention(ns) with interleaved PE filler
        # =================================================================
        ppool = octx.enter_context(tc.tile_pool(name="prep", bufs=1))
        bpool = octx.enter_context(tc.tile_pool(name="stageB", bufs=2))
        xT0, qT0 = new_prep_tiles()
        for r in range(4):
            emit_prep_transpose(r, xts0[r], xT0)
        emit_prep_q(xT0, qT0)
        qT_list = [None] * nsplits
        qT_list[0] = qT0
        outT_list = [None] * nsplits
        for ns in range(nsplits):
            qT = qT_list[ns]
            outT = [bpool.tile([128, 512], f32r, tag=f"oT{c}", name=f"oT{c}")
                    for c in range(4)]
            # build the PE-filler queue for this split: proj of ns-1 and
            # prep of ns+1, emitted just-in-time between attention steps
            filler = []
            if ns >= 1:
                for r in range(4):
                    filler.append(('proj', r))
            if ns + 1 < nsplits:
                xTn, qTn = new_prep_tiles()
                qT_list[ns + 1] = qTn
                xts = emit_prep_dma(ns + 1)
                for r in range(4):
                    filler.append(('ptr', r))
                filler.append(('pq', 0))
            nf = len(filler)
            fidx = 0
            it = 0
            for hp in range(4):
                po = [ps.tile([HD + 1, 512], f32, tag="att", name=f"po{e_}",
                               bufs=3) for e_ in range(2)]
                for mc in range(NK // 128):
                    pss = ps.tile([128, 1024], f32, tag="sc")
                    for e in range(2):
                        nc.tensor.matmul(
                            pss[:, ts(e, 512)],
                            kT[hp][ts(e, 64), ts(mc, 128)],
                            qT[hp][ts(e, 64), :],
                            start=True, stop=True,
                            tile_position=(64 * e, 0))
                    ex = bpool.tile([128, 1024], f32r, tag="ex")
                    nc.scalar.activation(ex[:], pss[:], AF.Exp, scale=SCALE)
                    # just-in-time PE filler while ACT computes the exp
                    while fidx < nf and fidx * 32 <= it * nf:
                        kind, r = filler[fidx]
                        if kind == 'proj':
                            emit_proj_block(outT_list[ns - 1], ns - 1, r)
                        elif kind == 'ptr':
                            emit_prep_transpose(r, xts[r], xTn)
                        else:
                            emit_prep_q(xTn, qTn)
                        fidx += 1
                    it += 1
                    for e in range(2):
                        h = 2 * hp + e
                        nc.tensor.matmul(
                            po[e][:],
                            v_aug[mc][:, ts(h, HD + 1)],
                            ex[:, ts(e, 512)],
                            start=(mc == 0), stop=(mc == NK // 128 - 1))
                for e in range(2):
                    # evacuate the accumulator immediately so the PSUM slot
                    # frees for the next head pair; normalize off the copy
                    poc = bpool.tile([HD + 1, 512], f32, tag="poc")
                    nc.vector.tensor_copy(poc[:], po[e][:])
                    dsb = bpool.tile([1, 512], f32, tag="dsb")
                    nc.vector.tensor_copy(dsb[:], poc[HD:HD + 1, :])
                    recip = bpool.tile([1, 512], f32, tag="recip")
                    nc.vector.reciprocal_approx_fast(recip[:], dsb[:])
                    bc = bpool.tile([HD, 512], f32, tag="bc")
                    nc.gpsimd.partition_broadcast(bc[:], recip[:])
                    nc.vector.tensor_tensor(outT[hp][ts(e, 64), :],
                                            poc[0:HD, :], bc[:],
                                            op=ALU.mult)
            outT_list[ns] = outT
        for r in range(4):
            emit_proj_block(outT_list[nsplits - 1], nsplits - 1, r)

    nc.compile()
    return nc



# ---------------------------------------------------------------------------
# SPMD entry point: full inputs in, full output out. One batch element per
# NeuronCore (B=8 over 8 cores), no collectives.
# ---------------------------------------------------------------------------
import numpy as np

_NC_CACHE = None


def _get_nc():
    global _NC_CACHE
    if _NC_CACHE is None:
        _NC_CACHE = build_core_program()
    return _NC_CACHE


_SHARED = ["Wq", "Wkv", "Wproj", "bproj", "g_cross", "b_cross", "sr_w",
           "sr_b", "g_sr", "b_sr"]


def _run(inputs, trace=False, trace_kwargs=None):
    from concourse.bass_utils import run_bass_kernel_spmd
    nc = _get_nc()
    shared = {k: np.ascontiguousarray(np.asarray(inputs[k], dtype=np.float32))
              for k in _SHARED}
    x = np.asarray(inputs["x"], dtype=np.float32)
    y = np.asarray(inputs["y"], dtype=np.float32)
    n_cores = 8
    in_maps = []
    for i in range(n_cores):
        m = dict(shared)
        m["x"] = np.ascontiguousarray(x[i])
        m["y"] = np.ascontiguousarray(y[i])
        in_maps.append(m)
    kw = {}
    if trace:
        kw["trace"] = True
        if trace_kwargs:
            kw.update(trace_kwargs)
    res = run_bass_kernel_spmd(nc, in_maps, list(range(n_cores)), **kw)
    out = np.stack([res.results[i]["out"] for i in range(n_cores)], axis=0)
    return out, res


def kernel(**inputs):
    out, _ = _run(inputs)
    return out
